# revision 3
# baseline (speedup 1.0000x reference)
"""Trainium2 Bass kernel for nn_CrossAttention_61735859912918.

B=4, SQ=SKV=2048, D=768, H=12, HD=16 (DR=192).
Sharding: 8 cores = (batch b, SQ half) -> each core computes a [1024, 768]
output slice independently (no collectives; K/V recomputed per SQ-half).

Per-core pipeline (all layouts chosen so softmax reduction lands on the
matmul contraction axis, never on partitions):
  - xT/encT loaded via DMA-transpose (bf16, host-cast)
  - QT/KT in "strip" layout: head h of group g at partitions 32*(h%4)..+16,
    so the K=16 score matmuls auto-pack 4 concurrent PE row-tiles.
  - scoresT[kv, q] waves of 3 heads -> one [128, 1536] exp on ACT (the
    bottleneck engine) -> attn@V with [v_h | 1] augmented rhs so the
    softmax denominator falls out of the same matmul.
  - normalize via free-dim broadcast, PE-transpose AO, project with Wp.
"""

import sys

sys.path.insert(0, "/opt/trn_rl_repo")

import numpy as np
import ml_dtypes
from contextlib import ExitStack

import concourse.bass as bass
import concourse.mybir as mybir
import concourse.tile as tile
from concourse import bacc
from concourse.bass import ds, ts
from concourse.masks import make_identity
from concourse import bass_utils

F32 = mybir.dt.float32
BF16 = mybir.dt.bfloat16
AF = mybir.ActivationFunctionType
ALU = mybir.AluOpType

B, SQ_FULL, SKV_FULL, D = 4, 2048, 2048, 768
H, HD, DR = 12, 16, 192
P = 128
KO = D // P  # 6

# waves of 3 heads; within each wave the strip indices j=h%4 are distinct so
# the three K=16 score matmuls land on distinct PE row-groups (concurrent).
WAVES = [
    [(0, 0), (0, 1), (0, 2)],
    [(0, 3), (1, 0), (1, 1)],
    [(1, 2), (1, 3), (2, 0)],
    [(2, 1), (2, 2), (2, 3)],
]


def build_program(SQL, SKV, debug_outs=False):
    """Emit the per-core Bass program. SQL = local q rows, SKV = kv rows."""
    assert SQL % P == 0 and SKV % P == 0
    QCH = min(512, SQL)      # q columns per score wave
    NQC = SQL // QCH
    NS4 = QCH // P           # q subtiles per chunk (4 full-size)
    NK = (NS4 + 1) // 2      # av psum tiles per chunk
    KVC = SKV // P
    NQS = SQL // P

    nc = bacc.Bacc("TRN2", target_bir_lowering=False, debug=False)

    x_ap = nc.dram_tensor("x", [SQL, D], BF16, kind="ExternalInput").ap()
    enc_ap = nc.dram_tensor("enc", [SKV, D], BF16, kind="ExternalInput").ap()
    wq_ap = nc.dram_tensor("wq_arr", [D, 3, P], BF16, kind="ExternalInput").ap()
    wk_ap = nc.dram_tensor("wk_arr", [D, 3, P], BF16, kind="ExternalInput").ap()
    wv_ap = nc.dram_tensor("wv", [D, DR], BF16, kind="ExternalInput").ap()
    wp0_ap = nc.dram_tensor("wp0", [P, D], BF16, kind="ExternalInput").ap()
    wp1_ap = nc.dram_tensor("wp1", [64, D], BF16, kind="ExternalInput").ap()
    bq_ap = nc.dram_tensor("bq_strip", [3, P, 1], F32, kind="ExternalInput").ap()
    bk_ap = nc.dram_tensor("bk_strip", [3, P, 1], F32, kind="ExternalInput").ap()
    bv_ap = nc.dram_tensor("bv_row", [1, DR], F32, kind="ExternalInput").ap()
    bp_ap = nc.dram_tensor("bp_row", [1, D], F32, kind="ExternalInput").ap()
    out_ap = nc.dram_tensor("out", [SQL, D], F32, kind="ExternalOutput").ap()
    if debug_outs:
        dbg_e = nc.dram_tensor("dbg_e", [P, 3 * min(512, SQL)], BF16,
                               kind="ExternalOutput").ap()
        dbg_ao = nc.dram_tensor("dbg_ao", [P, SQL // P, DR], BF16,
                                kind="ExternalOutput").ap()
        dbg_av = nc.dram_tensor("dbg_av", [P, 2, 204], F32,
                                kind="ExternalOutput").ap()

    with TileCtx(nc) as tc, ExitStack() as ctx:
        persist = ctx.enter_context(tc.tile_pool(name="persist", bufs=1))
        esb = ctx.enter_context(tc.tile_pool(name="esb", bufs=3))
        npool = ctx.enter_context(tc.tile_pool(name="npool", bufs=4))
        tailp = ctx.enter_context(tc.tile_pool(name="tailp", bufs=2))
        osbp = ctx.enter_context(tc.tile_pool(name="osbp", bufs=3))

        identb = persist.tile([P, P], BF16, name="identb", tag="identb")
        make_identity(nc, identb)

        # ---- load weights / biases ----
        wq_sb = persist.tile([P, KO, 3, P], BF16, name="wq", tag="wq")
        nc.sync.dma_start(wq_sb, wq_ap.rearrange("(ko p) g m -> p ko g m", p=P))
        wk_sb = persist.tile([P, KO, 3, P], BF16, name="wk", tag="wk")
        nc.sync.dma_start(wk_sb, wk_ap.rearrange("(ko p) g m -> p ko g m", p=P))
        wv_sb = persist.tile([P, KO, DR], BF16, name="wv", tag="wv")
        nc.sync.dma_start(wv_sb, wv_ap.rearrange("(ko p) n -> p ko n", p=P))
        wp0_sb = persist.tile([P, D], BF16, name="wp0", tag="wp0")
        nc.sync.dma_start(wp0_sb, wp0_ap)
        wp1_sb = persist.tile([64, D], BF16, name="wp1", tag="wp1")
        nc.sync.dma_start(wp1_sb, wp1_ap)
        bq_sb = persist.tile([P, 3, 1], F32, name="bq", tag="bq")
        nc.sync.dma_start(bq_sb, bq_ap.rearrange("g p one -> p g one"))
        bk_sb = persist.tile([P, 3, 1], F32, name="bk", tag="bk")
        nc.sync.dma_start(bk_sb, bk_ap.rearrange("g p one -> p g one"))
        bv_rep = persist.tile([P, DR], F32, name="bv", tag="bv")
        nc.gpsimd.dma_start(bv_rep, bv_ap.to_broadcast((P, DR)))
        bp_rep = persist.tile([P, D], F32, name="bp", tag="bp")
        nc.gpsimd.dma_start(bp_rep, bp_ap.to_broadcast((P, D)))

        # ---- transposing loads of x / enc, per (ko, chunk) tile so the
        # ---- projections unblock as soon as their slice lands ----
        NKC = SKV // QCH
        xt_t = [[persist.tile([P, QCH], BF16, name=f"xt{ko}_{qc}", tag=f"xt{ko}_{qc}")
                 for qc in range(NQC)] for ko in range(KO)]
        enct_t = [[persist.tile([P, QCH], BF16, name=f"et{ko}_{kc}", tag=f"et{ko}_{kc}")
                   for kc in range(NKC)] for ko in range(KO)]
        for kc in range(max(NKC, NQC)):
            for ko in range(KO):
                if kc < NKC:
                    nc.sync.dma_start_transpose(
                        enct_t[ko][kc], enc_ap[ds(kc * QCH, QCH), ds(ko * P, P)])
                if kc < NQC:
                    nc.sync.dma_start_transpose(
                        xt_t[ko][kc], x_ap[ds(kc * QCH, QCH), ds(ko * P, P)])

        qt_sb = [[persist.tile([P, QCH], BF16, name=f"qt{g}_{qc}", tag=f"qt{g}_{qc}")
                  for qc in range(NQC)] for g in range(3)]
        kt_sb = [[persist.tile([P, QCH], BF16, name=f"kt{g}_{kc}", tag=f"kt{g}_{kc}")
                  for kc in range(SKV // QCH)] for g in range(3)]
        va_t = [persist.tile([P, H, 17], BF16, name=f"va{c}", tag=f"va{c}")
                for c in range(KVC)]
        for c in range(KVC):
            nc.vector.memset(va_t[c][:, :, 16:17], 1.0)
        ao_sb = persist.tile([P, NQS, DR], BF16, name="ao", tag="ao")

        # ---- QT / KT / V projections, in attention-priority order ----
        def emit_qt(g, qc):
            ps = projp.tile([P, QCH], F32, name="proj", tag="proj")
            for ko in range(KO):
                nc.tensor.matmul(ps, wq_sb[:, ko, g, :], xt_t[ko][qc],
                                 start=(ko == 0), stop=(ko == KO - 1))
            nc.vector.tensor_scalar(qt_sb[g][qc], ps, bq_sb[:, g, :], None, ALU.add)

        def emit_kt(g, kc):
            ps = projp.tile([P, QCH], F32, name="proj", tag="proj")
            for ko in range(KO):
                nc.tensor.matmul(ps, wk_sb[:, ko, g, :], enct_t[ko][kc],
                                 start=(ko == 0), stop=(ko == KO - 1))
            nc.vector.tensor_scalar(kt_sb[g][kc], ps, bk_sb[:, g, :], None, ALU.add)

        def emit_v(c):
            ps = projp.tile([P, DR], F32, name="vproj", tag="vproj")
            for ko in range(KO):
                nc.tensor.matmul(
                    ps, enct_t[ko][c * P // QCH][:, ds((c * P) % QCH, P)],
                    wv_sb[:, ko, :], start=(ko == 0), stop=(ko == KO - 1))
            nc.vector.tensor_tensor(
                va_t[c][:, :, 0:16],
                ps.rearrange("p (h s) -> p h s", s=16),
                bv_rep.rearrange("p (h s) -> p h s", s=16),
                ALU.add,
            )

        CPK = QCH // P  # kv chunks per kt tile
        with tc.tile_pool(name="projp", bufs=2, space="PSUM") as projp:
            for g in range(3):
                emit_qt(g, 0)
            for g in range(3):
                emit_kt(g, 0)
            for c in range(CPK):
                emit_v(c)
            for qc in range(1, NQC):
                for g in range(3):
                    emit_qt(g, qc)
            for kc in range(1, NKC):
                for g in range(3):
                    emit_kt(g, kc)
                for c in range(kc * CPK, (kc + 1) * CPK):
                    emit_v(c)

        # ---- main attention loop ----
        with tc.tile_pool(name="spsum", bufs=2, space="PSUM") as spsum, \
             tc.tile_pool(name="avpsum", bufs=1, space="PSUM") as avpsum:
            for qc in range(NQC):
                avs = [avpsum.tile([P, 2, 204], F32, name=f"av{k}", tag=f"av{k}") for k in range(NK)]
                for c in range(KVC):
                    for w, wave in enumerate(WAVES):
                        sp = spsum.tile([P, 3 * QCH], F32, name="s", tag="s")
                        for i, (g, j) in enumerate(wave):
                            nc.tensor.matmul(
                                sp[:, ds(i * QCH, QCH)],
                                kt_sb[g][c * P // QCH][32 * j:32 * j + 16,
                                    ds((c * P) % QCH, P)],
                                qt_sb[g][qc][32 * j:32 * j + 16, :],
                                start=True, stop=True,
                                tile_position=(32 * j, 0),
                            )
                        e = esb.tile([P, 3 * QCH], BF16, name="e", tag="e")
                        nc.scalar.activation(e, sp, AF.Exp)
                        if debug_outs and qc == 0 and c == 0 and wave is WAVES[0]:
                            nc.sync.dma_start(dbg_e, e)
                        for i, (g, j) in enumerate(wave):
                            h = 4 * g + j
                            for s4 in range(NS4):
                                k, kk = divmod(s4, 2)
                                # one accumulation group per av PSUM bank:
                                # start marks the whole 2KB zero-region, so
                                # only the first mm into the tile may start
                                # and only the last may stop.
                                first = (c == 0 and w == 0 and i == 0
                                         and s4 == 2 * k)
                                last = (c == KVC - 1 and w == 3 and i == 2
                                        and s4 == min(2 * k + 1, NS4 - 1))
                                nc.tensor.matmul(
                                    avs[k][:, kk, ds(17 * h, 17)],
                                    e[:, ds(i * QCH + s4 * P, P)],
                                    va_t[c][:, h, :],
                                    start=first, stop=last,
                                    skip_group_check=True,
                                )
                if debug_outs and qc == 0:
                    dbg_av_sb = npool.tile([P, 2, 204], F32, name="dbgavsb", tag="dbgavsb")
                    nc.vector.tensor_copy(dbg_av_sb, avs[0])
                    nc.sync.dma_start(dbg_av, dbg_av_sb)
                # normalize: ao = av[:, :, :16] * (1 / av[:, :, 16])
                for k in range(NK):
                    avr = avs[k].rearrange("p kk (h s) -> p kk h s", s=17)
                    zr = npool.tile([P, 2, H, 1], F32, name="zr", tag="zr")
                    nc.vector.reciprocal(zr[:, :, :, 0], avr[:, :, :, 16])
                    for kk in range(2):
                        s4 = 2 * k + kk
                        if s4 >= NS4:
                            break
                        qs = qc * NS4 + s4
                        nc.vector.tensor_tensor(
                            ao_sb[:, qs, :].rearrange("p (h s) -> p h s", s=16),
                            avr[:, kk, :, 0:16],
                            zr[:, kk, :, :].to_broadcast((P, H, 16)),
                            ALU.mult,
                        )

        if debug_outs:
            nc.sync.dma_start(dbg_ao, ao_sb)

        # ---- tail: transpose AO, project with Wp, add bp, store ----
        with tc.tile_pool(name="aotp", bufs=2, space="PSUM") as aotp, \
             tc.tile_pool(name="outp", bufs=2, space="PSUM") as outp:
            for qs in range(NQS):
                tp = aotp.tile([P, 256], BF16, name="aot", tag="aot")
                nc.tensor.transpose(tp[:, 0:P], ao_sb[:, qs, 0:P], identb)
                nc.tensor.transpose(tp[:64, P:256], ao_sb[:, qs, P:DR], identb)
                aot0 = tailp.tile([P, P], BF16, name="aot0", tag="aot0")
                nc.vector.tensor_copy(aot0, tp[:, 0:P])
                aot1 = tailp.tile([64, P], BF16, name="aot1", tag="aot1")
                nc.vector.tensor_copy(aot1, tp[:64, P:256])
                for n2 in range(2):
                    op = outp.tile([P, 384], F32, name=f"out{n2}", tag=f"out{n2}")
                    nc.tensor.matmul(op, aot0, wp0_sb[:, ds(n2 * 384, 384)],
                                     start=True, stop=False)
                    nc.tensor.matmul(op, aot1, wp1_sb[:, ds(n2 * 384, 384)],
                                     start=False, stop=True)
                    osb = osbp.tile([P, 384], F32, name=f"osb{n2}", tag=f"osb{n2}")
                    nc.vector.tensor_tensor(osb, op, bp_rep[:, ds(n2 * 384, 384)],
                                            ALU.add)
                    nc.sync.dma_start(out_ap[ds(qs * P, P), ds(n2 * 384, 384)], osb)

    nc.compile()
    return nc


def TileCtx(nc):
    return tile.TileContext(nc)


def prep_weights(Wq, bq, Wkv, bkv, Wp, bp):
    """Host-side weight prep: strip layouts, bf16 casts, 1/sqrt(HD) folding."""
    f = np.float32
    Wq = np.asarray(Wq, f)
    Wkv = np.asarray(Wkv, f)
    Wp = np.asarray(Wp, f)
    bq = np.asarray(bq, f)
    bkv = np.asarray(bkv, f)
    bp = np.asarray(bp, f)
    scale = 1.0 / np.sqrt(HD).astype(f)

    def strip_w(W, s):  # [768, 192] -> [768, 3, 128] with 16-in-32 strips
        arr = np.zeros((D, 3, 4, 32), f)
        arr[:, :, :, :16] = (W * s).reshape(D, 3, 4, 16)
        return arr.reshape(D, 3, P).astype(ml_dtypes.bfloat16)

    def strip_b(b, s):  # [192] -> [3, 128, 1]
        arr = np.zeros((3, 4, 32), f)
        arr[:, :, :16] = (b * s).reshape(3, 4, 16)
        return arr.reshape(3, P, 1)

    return {
        "wq_arr": strip_w(Wq, scale),
        "wk_arr": strip_w(Wkv[:, :DR], 1.0),
        "wv": Wkv[:, DR:].astype(ml_dtypes.bfloat16),
        "wp0": Wp[:P].astype(ml_dtypes.bfloat16),
        "wp1": Wp[P:].astype(ml_dtypes.bfloat16),
        "bq_strip": strip_b(bq, scale),
        "bk_strip": strip_b(bkv[:DR], 1.0),
        "bv_row": bkv[DR:].reshape(1, DR).astype(f),
        "bp_row": bp.reshape(1, D).astype(f),
    }


def make_in_maps(hidden_states, encoder_hidden_states, Wq, bq, Wkv, bkv, Wp, bp,
                 n_cores=8):
    """Shard full inputs into per-core in_maps. core i -> (b=i//2, half=i%2)."""
    hs = np.asarray(hidden_states, np.float32)
    enc = np.asarray(encoder_hidden_states, np.float32)
    w = prep_weights(Wq, bq, Wkv, bkv, Wp, bp)
    sql = SQ_FULL // 2
    in_maps = []
    for i in range(n_cores):
        b, half = divmod(i, 2)
        m = dict(w)
        m["x"] = hs[b, half * sql:(half + 1) * sql].astype(ml_dtypes.bfloat16)
        m["enc"] = enc[b].astype(ml_dtypes.bfloat16)
        in_maps.append(m)
    return in_maps


_PROGRAM_CACHE = {}


def get_program(SQL=SQ_FULL // 2, SKV=SKV_FULL):
    key = (SQL, SKV)
    if key not in _PROGRAM_CACHE:
        _PROGRAM_CACHE[key] = build_program(SQL, SKV)
    return _PROGRAM_CACHE[key]


def kernel(hidden_states, encoder_hidden_states, Wq, bq, Wkv, bkv, Wp, bp,
           **run_kwargs):
    nc = get_program()
    in_maps = make_in_maps(hidden_states, encoder_hidden_states,
                           Wq, bq, Wkv, bkv, Wp, bp)
    res = bass_utils.run_bass_kernel_spmd(nc, in_maps, core_ids=list(range(8)),
                                          **run_kwargs)
    sql = SQ_FULL // 2
    out = np.empty((B, SQ_FULL, D), np.float32)
    for i in range(8):
        b, half = divmod(i, 2)
        out[b, half * sql:(half + 1) * sql] = res.results[i]["out"]
    if run_kwargs:
        kernel.last_results = res
    return out

